# revision 1
# baseline (speedup 1.0000x reference)
"""Cross-attention Trainium2 kernel (nn_CrossAttention, B=2, L=2048, D=1024,
Dctx=768, 16 heads x 64).

Sharding: 8 cores = 2 (batch) x 4 (head-groups of 4 heads). Each core computes
its batch's Q/K/V projections for its 4 heads, flash-style attention in the
transposed (S^T) domain, and a partial output projection; the host sums the
head-group partials and adds b_o.

Design notes (CoreSim-profiled, HW-verified; ~184us/core vs 269us baseline):
- All activations live transposed on-chip (xT, ctxT, qT, kT, attnT): every
  matmul contracts over the partition dim, no on-chip transposes. Host ships
  x/context/w_q/w_k/w_v pre-transposed/sliced in bf16 (halves input DMA;
  ~4e-3 rel err, gate is 2e-2); scores/AV/out-proj run in float32r.
- Softmax denominator: 32 ones-columns lead each head's 96-wide lane of v_t
  ([ones|v] order, memset once) so the AV matmul emits a replicated d-block
  at PSUM partitions 0:32 -- read in place by the single-op fast reciprocal.
  Custom-DVE ops silently mis-read nonzero base partitions on HW (CoreSim
  models them correctly), so the d-block MUST sit at partition 0.
- One shared 4-slot PSUM pool (2 banks/slot). AV lags 3 iterations behind
  S/exp (software pipeline) so the score->exp chain never stalls on the
  previous head's normalize; exp tiles are 6-deep.
- Loops run half-outer, heads inner. The half-0 output projection drips one
  matmul per j through half-1's PE slack; the deferred Q(s2,s3) projections
  and K(s2,s3) drip through h0-h2 of half 0 the same way, which pulls the
  first exp to ~20us. ACT (exp, 133us busy) and PE (157us busy) are the
  co-binding engines; the attention j-loop is exp-paced at 1038ns/tile.
- Tail: [128,1024] out-proj slabs, copies alternating ACT/DVE, bf16 outT
  (halves output DMA). Strided multi-dim memsets are avoided (2D only).
"""
import numpy as np

import concourse.bass as bass
import concourse.tile as tile
from concourse import bacc, mybir, bass_utils

F32R = mybir.dt.float32r
F32 = mybir.dt.float32
BF16 = mybir.dt.bfloat16
EXP = mybir.ActivationFunctionType.Exp
IDENT = mybir.ActivationFunctionType.Identity

# Problem shape (hardcoded per harness contract)
B, LQ, D = 2, 2048, 1024
DCTX = 768
NH, HD = 16, 64
SCALE = 1.0 / 8.0  # 1/sqrt(64)

# Per-core shard: 4 heads (one group), one batch
GH = 4                # heads per core
ONES = 32             # d-replication rows per head
VW = HD + ONES        # 96: per-head width in v_t
VAW = GH * VW         # 384
KT_Q = D // 128       # 8
KT_C = DCTX // 128    # 6
NLK = LQ // 128       # 16 key tiles
NS = LQ // 512        # 4 query 512-slices
HALF = 1024


def _build():
    nc = bacc.Bacc("TRN2", target_bir_lowering=False, debug=False,
                   enable_asserts=False, num_devices=8)

    xT_d = nc.dram_tensor("xT", (D, LQ), BF16, kind="ExternalInput").ap()
    cT_d = nc.dram_tensor("ctxT", (DCTX, LQ), BF16, kind="ExternalInput").ap()
    wq_d = nc.dram_tensor("wq", (D, 256), BF16, kind="ExternalInput").ap()
    wk_d = nc.dram_tensor("wk", (DCTX, 256), BF16, kind="ExternalInput").ap()
    wv_d = nc.dram_tensor("wv", (DCTX, 256), BF16, kind="ExternalInput").ap()
    wo_d = nc.dram_tensor("wo", (256, D), F32R, kind="ExternalInput").ap()
    bq_d = nc.dram_tensor("bq", (128, 2), F32, kind="ExternalInput").ap()
    bk_d = nc.dram_tensor("bk", (128, 2), F32, kind="ExternalInput").ap()
    bv_d = nc.dram_tensor("bv", (128, 256), F32, kind="ExternalInput").ap()
    out_d = nc.dram_tensor("outT", (D, LQ), BF16, kind="ExternalOutput").ap()

    with tile.TileContext(nc) as tc:
        with tc.tile_pool(name="w", bufs=1) as wp, \
             tc.tile_pool(name="xt", bufs=4) as xtp, \
             tc.tile_pool(name="ct", bufs=4) as ctp, \
             tc.tile_pool(name="act", bufs=1) as actp, \
             tc.tile_pool(name="expp", bufs=6) as expp, \
             tc.tile_pool(name="rdp", bufs=3) as rdp, \
             tc.tile_pool(name="outp", bufs=6) as outp, \
             tc.tile_pool(name="psu", bufs=4, space="PSUM") as psu:

            # ---- K-projection dependencies first (PE starts ~4us in) ----
            wk_t = wp.tile([128, KT_C * 256], BF16, tag="wk")
            nc.sync.dma_start(wk_t[:].rearrange("p (kt m) -> p kt m", m=256),
                              wk_d.rearrange("(kt p) m -> p kt m", p=128))
            bk_t = wp.tile([128, 2], F32, tag="bk")
            nc.sync.dma_start(bk_t[:], bk_d[:])

            # ---- persistent activation tiles ----
            qT = [actp.tile([128, LQ], F32R, tag=f"qT{p}", name=f"qT{p}")
                  for p in range(2)]
            kT = [actp.tile([128, LQ], F32R, tag=f"kT{p}", name=f"kT{p}")
                  for p in range(2)]
            v_t = actp.tile([128, NLK * VAW], F32R, tag="v")
            aT = [actp.tile([128, LQ], F32R, tag=f"aT{p}", name=f"aT{p}")
                  for p in range(2)]

            # ones columns of v_t (softmax denominator rows), set once.
            # Contiguous 2D memsets only -- strided multi-dim memset
            # mislowers on HW (passes CoreSim).
            for _j in range(NLK):
                for _h in range(GH):
                    _off = VAW * _j + VW * _h
                    nc.vector.memset(v_t[:, _off:_off + ONES].bitcast(F32), 1.0)

            # ---- K + Q projections interleaved per 512-slice; Q for s=2,3 is
            # deferred into the h0 attention loop (only needed at half 1) ----
            wq_t = wp.tile([128, KT_Q * 256], BF16, tag="wq")
            bq_t = wp.tile([128, 2], F32, tag="bq")
            wv_t = wp.tile([128, KT_C * 256], BF16, tag="wv")
            bv_t = wp.tile([128, 256], F32, tag="bv")
            wo_t = wp.tile([128, 2 * D], F32R, tag="wo")
            ct_tiles = {}

            def k_proj(s, p_sel=(0, 1)):
                if p_sel[0] == 0:
                    t = ctp.tile([128, KT_C * 512], BF16, tag="ct")
                    tv = t[:].rearrange("p (kt q) -> p kt q", q=512)
                    cv = cT_d.rearrange("(kt p) q -> p kt q", p=128)[:, :,
                                                                    512 * s:512 * (s + 1)]
                    # chunked so the consuming matmuls start sooner
                    for kk in range(3):
                        nc.sync.dma_start(tv[:, 2 * kk:2 * kk + 2, :],
                                          cv[:, 2 * kk:2 * kk + 2, :])
                    ct_tiles[s] = t
                t = ct_tiles[s]
                for p in p_sel:
                    ps = psu.tile([128, 512], F32, tag="u")
                    for kt in range(KT_C):
                        nc.tensor.matmul(
                            ps[:], wk_t[:, 256 * kt + 128 * p:256 * kt + 128 * (p + 1)],
                            t[:, 512 * kt:512 * (kt + 1)],
                            start=(kt == 0), stop=(kt == KT_C - 1))
                    nc.vector.tensor_scalar_add(
                        kT[p][:, 512 * s:512 * (s + 1)], ps[:], bk_t[:, p:p + 1])

            def xt_dma(s):
                t = xtp.tile([128, KT_Q * 512], BF16, tag="xt")
                tv = t[:].rearrange("p (kt q) -> p kt q", q=512)
                xv = xT_d.rearrange("(kt p) q -> p kt q", p=128)[:, :,
                                                                512 * s:512 * (s + 1)]
                for kk in range(2):
                    nc.sync.dma_start(tv[:, 4 * kk:4 * kk + 4, :],
                                      xv[:, 4 * kk:4 * kk + 4, :])
                xt_tiles[s] = t

            def q_mm(s, p, ps, kts):
                t = xt_tiles[s]
                for kt in kts:
                    nc.tensor.matmul(
                        ps[:], wq_t[:, 256 * kt + 128 * p:256 * kt + 128 * (p + 1)],
                        t[:, 512 * kt:512 * (kt + 1)],
                        start=(kt == 0), stop=(kt == KT_Q - 1))

            def q_add(s, p, ps):
                nc.vector.tensor_scalar_add(
                    qT[p][:, 512 * s:512 * (s + 1)], ps[:], bq_t[:, p:p + 1])

            def q_proj(s, p_sel=(0, 1)):
                if p_sel[0] == 0:
                    xt_dma(s)
                for p in p_sel:
                    ps = psu.tile([128, 512], F32, tag="u")
                    q_mm(s, p, ps, range(KT_Q))
                    q_add(s, p, ps)

            # pre-attention: K(s0,s1) + Q(s0,s1); K(s2,s3) burst inside h0's
            # j-loop before their key blocks are needed (cuts the DMA-bound
            # lead-in to the first exp)
            xt_tiles = {}
            k_proj(0)
            nc.sync.dma_start(wq_t[:].rearrange("p (kt m) -> p kt m", m=256),
                              wq_d.rearrange("(kt p) m -> p kt m", p=128))
            nc.sync.dma_start(bq_t[:], bq_d[:])
            k_proj(1)
            q_proj(0, p_sel=(0,))
            q_proj(0, p_sel=(1,))
            q_proj(1, p_sel=(0,))
            nc.sync.dma_start(wv_t[:].rearrange("p (kt m) -> p kt m", m=256),
                              wv_d.rearrange("(kt p) m -> p kt m", p=128))
            nc.sync.dma_start(bv_t[:], bv_d[:])
            nc.sync.dma_start(wo_t[:].rearrange("p (p2 m) -> p p2 m", m=1024),
                              wo_d.rearrange("(p2 p) m -> p p2 m", p=128))

            def v_chunk(j):
                # V projection for key block j: [128 keys, 256] -> strided
                # 64-wide head lanes of v_t (ones columns pre-set).
                ps = psu.tile([128, 256], F32, tag="u")
                s, jj = j // 4, j % 4
                for kt in range(KT_C):
                    nc.tensor.matmul(
                        ps[:],
                        ct_tiles[s][:, 512 * kt + 128 * jj:512 * kt + 128 * (jj + 1)],
                        wv_t[:, 256 * kt:256 * (kt + 1)],
                        start=(kt == 0), stop=(kt == KT_C - 1))
                for hh in range(GH):
                    o = VAW * j + VW * hh + ONES
                    nc.vector.tensor_add(
                        v_t[:, o:o + HD],
                        ps[:, HD * hh:HD * (hh + 1)],
                        bv_t[:, HD * hh:HD * (hh + 1)])

            def out_mm(ps_ap, mo, s, p):
                nc.tensor.matmul(
                    ps_ap, wo_t[:, D * p + 128 * mo:D * p + 128 * (mo + 1)],
                    aT[p][:, 512 * s:512 * (s + 1)],
                    start=(p == 0), stop=(p == 1))

            def out_emit(ps, mo, s, on_act=False):
                ot = outp.tile([128, 512], BF16, tag="out")
                if on_act:
                    nc.scalar.copy(ot[:], ps[:])
                else:
                    nc.vector.tensor_copy(ot[:], ps[:])
                nc.sync.dma_start(
                    out_d[128 * mo:128 * (mo + 1), 512 * s:512 * (s + 1)], ot[:])

            def out_proj(mo, s, on_act=False):
                ps = psu.tile([128, 512], F32, tag="u")
                out_mm(ps[:], mo, s, 0)
                out_mm(ps[:], mo, s, 1)
                out_emit(ps, mo, s, on_act)

            # ---- attention: half outer, heads inner ----
            for half in range(2):
                for h in range(GH):
                    p, m = h // 2, h % 2
                    r0 = 64 * m
                    pa = psu.tile([96, HALF], F32, tag="u")
                    drip = None
                    qps = None
                    exq = []

                    def av(j, ex, pa=pa, h=h):
                        for n in range(2):
                            nc.tensor.matmul(
                                pa[:, 512 * n:512 * (n + 1)],
                                v_t[:, VAW * j + VW * h:VAW * j + VW * h + VW],
                                ex[:, 512 * n:512 * (n + 1)],
                                start=(j == 0), stop=(j == NLK - 1))

                    for j in range(NLK):
                        if half == 0:
                            if h == 0:
                                if j in (3, 7):
                                    v_chunk(j)
                                    v_chunk(j + 1)
                                elif j in (4, 8):
                                    k_proj(2 + (j == 8), p_sel=(0,))
                                elif j in (5, 9):
                                    k_proj(2 + (j == 9), p_sel=(1,))
                                    v_chunk(j)
                                else:
                                    v_chunk(j)
                            # deferred Q proj (s=2,3) dripped 2 matmuls per j
                            # through h1/h2 PE slack
                            else:
                                # two dripped 8-matmul proj streams per head:
                                # h1: Q(s0,p1)+Q(s1,p1); h2: Q(s3); h3: Q(s2)
                                sa, sb, pp = ((1, None, 1), (3, 3, None),
                                              (2, 2, None))[h - 1]
                                if j == 0 and h == 2:
                                    xt_dma(3)
                                elif j == 2 and h == 2:
                                    xt_dma(2)
                                elif j in (4, 9):
                                    qs = sa if j == 4 else sb
                                    if qs is not None:
                                        qp = pp if pp is not None else (j == 9)
                                        qp = int(qp)
                                        qps = psu.tile([128, 512], F32, tag="u")
                                        q_mm(qs, qp, qps, (0, 1))
                                elif j in (5, 6, 7, 10, 11, 12):
                                    qs = sa if j < 8 else sb
                                    if qs is not None:
                                        qp = pp if pp is not None else (j >= 8)
                                        qp = int(qp)
                                        k0 = 2 * (j - (4 if j < 8 else 9))
                                        q_mm(qs, qp, qps, (k0, k0 + 1))
                                elif j in (8, 13):
                                    qs = sa if j == 8 else sb
                                    if qs is not None:
                                        qp = pp if pp is not None else (j == 13)
                                        q_add(qs, int(qp), qps)
                                        qps = None
                        # drip half-0 out-projection through half-1 j-loops,
                        # one matmul per j, clear of head boundaries
                        elif h < 3 and j in (1, 3, 5, 7, 9, 11) and (h, j) not in ((0, 1), (0, 3)):
                            ti = {0: {5: 0, 7: 1, 9: 2, 11: 3},
                                  1: {1: 4, 3: 5, 5: 6, 7: 7, 9: 8, 11: 9},
                                  2: {1: 10, 3: 11, 5: 12, 7: 13, 9: 14,
                                      11: 15}}[h][j]
                            mo, sd = ti // 2, ti % 2
                            ps_d = psu.tile([128, 512], F32, tag="u")
                            drip = (ps_d, mo, sd)
                            out_mm(ps_d[:], mo, sd, 0)
                        elif drip is not None and h < 3 and j in (2, 4, 6, 8, 10, 12):
                            ps_d, mo, sd = drip
                            drip = None
                            out_mm(ps_d[:], mo, sd, 1)
                            out_emit(ps_d, mo, sd)
                        st = psu.tile([128, HALF], F32, tag="u")
                        for n in range(2):
                            nc.tensor.matmul(
                                st[:, 512 * n:512 * (n + 1)],
                                kT[p][r0:r0 + 64, 128 * j:128 * (j + 1)],
                                qT[p][r0:r0 + 64,
                                      HALF * half + 512 * n:HALF * half + 512 * (n + 1)],
                                start=True, stop=True)
                        ex = expp.tile([128, HALF], F32R, tag="expS")
                        nc.scalar.activation(ex[:], st[:], EXP, scale=SCALE)
                        exq.append((j, ex))
                        # AV lags 2 iterations so scores keep flowing while pa
                        # waits on the previous head's normalize
                        if len(exq) > 3:
                            av(*exq.pop(0))
                    while exq:
                        av(*exq.pop(0))
                    # normalize: attnT = attnU * (1/d), per 512-slice so the
                    # tail out-projection can start on the first slice early
                    rd = rdp.tile([ONES, HALF], F32, tag="rd")
                    # d-block rides at pa partitions 0:32 ([ones|v] lane
                    # layout) -- custom-DVE ops need base partition 0 on HW
                    nc.vector.reciprocal_approx_fast(rd[:], pa[0:32, :])
                    for n in range(2):
                        cs = slice(HALF * half + 512 * n, HALF * half + 512 * (n + 1))
                        ns_ = slice(512 * n, 512 * (n + 1))
                        nc.vector.tensor_mul(aT[p][r0:r0 + 32, cs], pa[32:64, ns_],
                                             rd[:, ns_])
                        nc.vector.tensor_mul(aT[p][r0 + 32:r0 + 64, cs],
                                             pa[64:96, ns_], rd[:, ns_])

            # ---- tail: half-1 output projection, [128,1024] slabs ----
            for mo in range(D // 128):
                ps = psu.tile([128, HALF], F32, tag="u")
                for n, s in enumerate((2, 3)):
                    out_mm(ps[:, 512 * n:512 * (n + 1)], mo, s, 0)
                    out_mm(ps[:, 512 * n:512 * (n + 1)], mo, s, 1)
                ot = outp.tile([128, HALF], BF16, tag="out")
                if mo % 2 == 0:
                    nc.scalar.copy(ot[:], ps[:])
                else:
                    nc.vector.tensor_copy(ot[:], ps[:])
                nc.sync.dma_start(
                    out_d[128 * mo:128 * (mo + 1), HALF:LQ], ot[:])

    nc.compile()
    return nc


_NC_CACHE = []


def _get_nc():
    if not _NC_CACHE:
        _NC_CACHE.append(_build())
    return _NC_CACHE[0]


OUT_NAME = "outT"


def prep_maps(inputs):
    """Host-side prep: per-core input tensor maps."""
    import ml_dtypes
    bf16 = ml_dtypes.bfloat16
    x = np.asarray(inputs["x"], np.float32)
    context = np.asarray(inputs["context"], np.float32)
    w_q = np.asarray(inputs["w_q"], np.float32)
    b_q = np.asarray(inputs["b_q"], np.float32)
    w_k = np.asarray(inputs["w_k"], np.float32)
    b_k = np.asarray(inputs["b_k"], np.float32)
    w_v = np.asarray(inputs["w_v"], np.float32)
    b_v = np.asarray(inputs["b_v"], np.float32)
    w_o = np.asarray(inputs["w_o"], np.float32)

    xTb = [np.ascontiguousarray(x[b].T).astype(bf16) for b in range(B)]
    cTb = [np.ascontiguousarray(context[b].T).astype(bf16) for b in range(B)]
    maps = []
    for c in range(8):
        b, g = c // 4, c % 4
        hs = slice(256 * g, 256 * (g + 1))
        maps.append({
            "xT": xTb[b],
            "ctxT": cTb[b],
            "wq": np.ascontiguousarray(w_q[:, hs]).astype(bf16),
            "wk": np.ascontiguousarray(w_k[:, hs]).astype(bf16),
            "wv": np.ascontiguousarray(w_v[:, hs]).astype(bf16),
            "wo": np.ascontiguousarray(w_o[hs, :]),
            "bq": np.ascontiguousarray(b_q[hs].reshape(2, 128).T),
            "bk": np.ascontiguousarray(b_k[hs].reshape(2, 128).T),
            "bv": np.broadcast_to(b_v[None, hs], (128, 256)).copy(),
        })
    return maps


def ref_partial(inputs, c):
    """Numpy reference for core c's outT partial (no b_o)."""
    x = np.asarray(inputs["x"], np.float64)
    context = np.asarray(inputs["context"], np.float64)
    b, g = c // 4, c % 4
    hs = slice(256 * g, 256 * (g + 1))
    q = x[b] @ inputs["w_q"][:, hs].astype(np.float64) + inputs["b_q"][hs]
    k = context[b] @ inputs["w_k"][:, hs].astype(np.float64) + inputs["b_k"][hs]
    v = context[b] @ inputs["w_v"][:, hs].astype(np.float64) + inputs["b_v"][hs]
    out = np.zeros((LQ, 256))
    for h in range(GH):
        cs = slice(HD * h, HD * (h + 1))
        s = (q[:, cs] @ k[:, cs].T) * SCALE
        w = np.exp(s - s.max(axis=-1, keepdims=True))
        w /= w.sum(axis=-1, keepdims=True)
        out[:, cs] = w @ v[:, cs]
    return np.ascontiguousarray(
        (out @ inputs["w_o"][hs, :].astype(np.float64)).T).astype(np.float32)


def kernel_run(inputs, trace=False, **kw):
    """Run on HW; returns (full_output, BassKernelResults)."""
    b_o = np.asarray(inputs["b_o"], np.float32)
    maps = prep_maps(inputs)
    nc = _get_nc()
    res = bass_utils.run_bass_kernel_spmd(nc, maps, core_ids=list(range(8)),
                                          trace=trace, **kw)
    out = np.empty((B, LQ, D), np.float32)
    for b in range(B):
        acc = res.results[4 * b]["outT"].astype(np.float32)
        for g in range(1, 4):
            acc = acc + res.results[4 * b + g]["outT"]
        out[b] = acc.T + b_o[None, :]
    return out, res


def kernel(**inputs) -> np.ndarray:
    out, _ = kernel_run(inputs)
    return out

